# revision 12
# baseline (speedup 1.0000x reference)
"""Trainium2 Bass kernel for the ApproxCompressor problem.

Reference computation (per batch n):
  loudness = mean_c(x^2)                                 (N, L)
  env = causal FIR conv with h[k] = (1-a) a^k, a = sigmoid(z_alpha),
        truncated at 16384 taps (equals the full 1-pole IIR in f32:
        a^16384 underflows for any a reachable from normal z)
  log_energy = log(env + 1e-5)
  gain = exp((1/ratio - 1) * softplus(knee*(log_energy - T)) / knee)
  out = gain * x

Strategy: data-parallel over N across the 8 NeuronCores (4 batches per
core). Per core layout: partition p = n_local*32 + chunk (128 rows),
each row holds 4096 consecutive samples; channels stacked along the
free dim of a [128, 8192] tile. The IIR runs as a DVE tensor_tensor_scan
along the free dim per row; cross-row carries are injected via a tiny
128x128 power-matrix matmul on the TensorEngine (exact, and exactly
zero whenever a^4096 underflows). Gain chain runs on the ScalarEngine
using only Ln/Exp/Square (all in one activation-table set; softplus is
computed as Ln(Exp(u)+1)).

Scheduling constraint honored throughout: every compute instruction can
carry at most ONE semaphore wait, so ops are ordered such that each
introduces at most one producer its engine has not yet observed (DVE
"pre-touch" copies absorb DMA-queue waits before multi-input ops).
"""
import sys
import contextlib

import numpy as np

for _p in ("/opt/trn_rl_repo", "/root/.axon_site/_ro/trn_rl_repo"):
    if _p not in sys.path:
        sys.path.append(_p)

import concourse.bass as bass
import concourse.mybir as mybir
from concourse import tile
from concourse.tile import add_dep_helper
from concourse.bass_utils import run_bass_kernel_spmd

N, C, L = 32, 2, 131072
NCORES = 8
NB = N // NCORES         # batches per core = 4
NCHUNK = 32              # row-chunks per batch
P = NB * NCHUNK          # 128 partitions
B = L // NCHUNK          # 4096 samples per row
FC = 2048                # f-chunk size for pipelining
NF = B // FC             # 2 chunks

F32 = mybir.dt.float32
BF16 = mybir.dt.bfloat16
I32 = mybir.dt.int32
AF = mybir.ActivationFunctionType
OP = mybir.AluOpType

_NC_CACHE = None


def _build_nc():
    nc = bass.Bass()
    x_d = nc.declare_dram_parameter("input_signals", [NB, C, L], F32, isOutput=False)
    za_d = nc.declare_dram_parameter("z_alpha", [NB, 1], F32, isOutput=False)
    lt_d = nc.declare_dram_parameter("log_threshold", [NB, 1], F32, isOutput=False)
    lr_d = nc.declare_dram_parameter("log_ratio", [NB, 1], F32, isOutput=False)
    lk_d = nc.declare_dram_parameter("log_knee", [NB, 1], F32, isOutput=False)
    out_d = nc.declare_dram_parameter("out", [NB, C, L], F32, isOutput=True)

    with tile.TileContext(nc) as tc, contextlib.ExitStack() as ctx:
        pool = ctx.enter_context(tc.tile_pool(name="main", bufs=1))
        ppool = ctx.enter_context(tc.tile_pool(name="psum", bufs=1, space="PSUM"))

        # ---- gpsimd iotas first (one proc; later DVE waits once) ----
        miot = pool.tile([P, P], I32)            # p - p' - 1
        nc.gpsimd.iota(miot[:, :], [[1, P]], base=-1, channel_multiplier=-1)
        niot = pool.tile([P, NB, NCHUNK], I32)   # n(col)
        nc.gpsimd.iota(niot[:, :, :], [[1, NB], [0, NCHUNK]], base=0,
                       channel_multiplier=0)
        piot = pool.tile([P, 1], I32)            # p'
        piot_inst = nc.gpsimd.iota(piot[:, :], [[1, 1]], base=0,
                                   channel_multiplier=1)

        # ---- param DMAs: col[p] = param[p // NCHUNK] ----
        def rep_load(src, nm):
            dst = pool.tile([P, 1], F32, name=nm)
            bsrc = src[:, 0:1].unsqueeze(1).to_broadcast((NB, NCHUNK, 1))
            dma_insts.append(nc.sync.dma_start(out=dst[:, 0:1], in_=bsrc))
            return dst

        dma_insts = []

        zcol = rep_load(za_d, "zcol")
        ltcol = rep_load(lt_d, "ltcol")
        lrcol = rep_load(lr_d, "lrcol")
        lkcol = rep_load(lk_d, "lkcol")

        # ---- main input DMAs ----
        xt = pool.tile([P, C * B], F32)

        def xs(c, j):
            lo = c * B + j * FC
            return slice(lo, lo + FC)

        def fs(j):
            return slice(j * FC, (j + 1) * FC)

        x_view = [x_d[:, c, :].rearrange("n (k f) -> n k f", k=NCHUNK)
                  for c in range(C)]
        o_view = [out_d[:, c, :].rearrange("n (k f) -> n k f", k=NCHUNK)
                  for c in range(C)]

        for c in range(C):
            for j in range(NF):
                dma_insts.append(
                    nc.sync.dma_start(out=xt[:, xs(c, j)],
                                      in_=x_view[c][:, :, fs(j)]))

        # ---- DVE const memsets (no deps) ----
        cm1 = pool.tile([P, 1], F32)
        nc.vector.memset(cm1[:, :], -1.0)
        c1 = pool.tile([P, 1], F32)
        nc.vector.memset(c1[:, :], 1.0)
        ceps = pool.tile([P, 1], F32)
        nc.vector.memset(ceps[:, :], 1e-5)

        # ---- DVE pre-touches: absorb DMA-queue waits one at a time ----
        touches = {}
        for c in range(C):
            for j in range(NF):
                tch = pool.tile([P, 1], F32, name=f"touch_{c}_{j}")
                touches[(c, j)] = nc.vector.tensor_copy(
                    tch[:, :], xt[:, c * B + j * FC:c * B + j * FC + 1])

        # ---- derived per-partition scalars ----
        # ACT order: enz(waits za-dma), lnd(waits DVE), knee(waits lk-dma),
        # er(waits lr-dma) -- each one new wait.
        enz = pool.tile([P, 1], F32)
        nc.scalar.activation(out=enz[:, :], in_=zcol[:, :], func=AF.Exp, scale=-1.0)
        dno = pool.tile([P, 1], F32)          # 1 + e^-z
        nc.vector.tensor_scalar(out=dno[:, :], in0=enz[:, :], scalar1=1.0,
                                scalar2=None, op0=OP.add)
        alpha = pool.tile([P, 1], F32)        # sigmoid(z)
        nc.vector.reciprocal(alpha[:, :], dno[:, :])
        lnd = pool.tile([P, 1], F32)          # -ln(a) = softplus(-z)
        nc.scalar.activation(out=lnd[:, :], in_=dno[:, :], func=AF.Ln)
        lkm1 = pool.tile([P, 1], F32)         # lk - 1 (DVE; waits lk-dma)
        nc.vector.tensor_scalar(out=lkm1[:, :], in0=lkcol[:, :], scalar1=1.0,
                                scalar2=None, op0=OP.subtract)
        knee = pool.tile([P, 1], F32)         # exp(lk - 1)
        nc.scalar.activation(out=knee[:, :], in_=lkm1[:, :], func=AF.Exp)
        er = pool.tile([P, 1], F32)           # e^lr
        nc.scalar.activation(out=er[:, :], in_=lrcol[:, :], func=AF.Exp)

        tthr = pool.tile([P, 1], F32)         # T = lt - 6  (DVE; waits lt-dma)
        nc.vector.tensor_scalar(out=tthr[:, :], in0=ltcol[:, :], scalar1=6.0,
                                scalar2=None, op0=OP.subtract)
        kT = pool.tile([P, 1], F32)           # waits ACT (knee)
        nc.vector.tensor_tensor(out=kT[:, :], in0=knee[:, :], in1=tthr[:, :],
                                op=OP.mult)
        negkT = pool.tile([P, 1], F32)
        nc.vector.tensor_scalar(out=negkT[:, :], in0=kT[:, :], scalar1=-1.0,
                                scalar2=None, op0=OP.mult)
        ratio = pool.tile([P, 1], F32)        # 1 + e^lr (waits ACT er tick)
        nc.vector.tensor_scalar(out=ratio[:, :], in0=er[:, :], scalar1=1.0,
                                scalar2=None, op0=OP.add)
        invr = pool.tile([P, 1], F32)
        nc.vector.reciprocal(invr[:, :], ratio[:, :])
        invr1 = pool.tile([P, 1], F32)        # 1/ratio - 1
        nc.vector.tensor_scalar(out=invr1[:, :], in0=invr[:, :], scalar1=1.0,
                                scalar2=None, op0=OP.subtract)
        invknee = pool.tile([P, 1], F32)
        nc.vector.reciprocal(invknee[:, :], knee[:, :])
        gamma = pool.tile([P, 1], F32)        # (1/ratio - 1)/knee
        nc.vector.tensor_tensor(out=gamma[:, :], in0=invr1[:, :],
                                in1=invknee[:, :], op=OP.mult)
        sclloud = pool.tile([P, 1], F32)      # 0.5*(1 - a)
        nc.vector.tensor_scalar(out=sclloud[:, :], in0=alpha[:, :], scalar1=-0.5,
                                scalar2=0.5, op0=OP.mult, op1=OP.add)

        # ---- carry matrix G[p', p] = (a^B)^(p-p'-1), same batch, p'<p ----
        pf = pool.tile([P, 1], F32)           # waits gpsimd (covers all iotas)
        nc.vector.tensor_scalar(out=pf[:, :], in0=piot[:, :], scalar1=0.0,
                                scalar2=None, op0=OP.add)
        # n(p') = p' // NCHUNK as sum of step comparisons (no mod/shift ISA)
        ge1 = pool.tile([P, 1], F32)
        nc.vector.tensor_scalar(out=ge1[:, :], in0=pf[:, :],
                                scalar1=float(NCHUNK), scalar2=None, op0=OP.is_ge)
        ge2 = pool.tile([P, 1], F32)
        nc.vector.tensor_scalar(out=ge2[:, :], in0=pf[:, :],
                                scalar1=float(2 * NCHUNK), scalar2=None,
                                op0=OP.is_ge)
        ge3 = pool.tile([P, 1], F32)
        nc.vector.tensor_scalar(out=ge3[:, :], in0=pf[:, :],
                                scalar1=float(3 * NCHUNK), scalar2=None,
                                op0=OP.is_ge)
        g12 = pool.tile([P, 1], F32)
        nc.vector.tensor_tensor(out=g12[:, :], in0=ge1[:, :], in1=ge2[:, :],
                                op=OP.add)
        ncol = pool.tile([P, 1], F32)
        nc.vector.tensor_tensor(out=ncol[:, :], in0=g12[:, :], in1=ge3[:, :],
                                op=OP.add)
        eqn = pool.tile([P, P], F32)
        nc.vector.tensor_scalar(out=eqn[:, :],
                                in0=niot.rearrange("p a b -> p (a b)"),
                                scalar1=ncol[:, :], scalar2=None, op0=OP.is_equal)
        pen_n = pool.tile([P, P], F32)
        nc.vector.tensor_scalar(out=pen_n[:, :], in0=eqn[:, :], scalar1=1.0,
                                scalar2=1e30, op0=OP.subtract, op1=OP.mult)
        geq = pool.tile([P, P], F32)
        nc.vector.tensor_scalar(out=geq[:, :], in0=miot[:, :], scalar1=0.0,
                                scalar2=None, op0=OP.is_ge)
        pen_m = pool.tile([P, P], F32)
        nc.vector.tensor_scalar(out=pen_m[:, :], in0=geq[:, :], scalar1=1.0,
                                scalar2=1e30, op0=OP.subtract, op1=OP.mult)
        ee = pool.tile([P, P], F32)           # (M * lnd) * -B  (lnd: ACT tick)
        nc.vector.tensor_scalar(out=ee[:, :], in0=miot[:, :], scalar1=lnd[:, :],
                                scalar2=float(-B), op0=OP.mult, op1=OP.mult)
        ee2 = pool.tile([P, P], F32)
        nc.vector.tensor_tensor(out=ee2[:, :], in0=ee[:, :], in1=pen_n[:, :],
                                op=OP.add)
        ee3 = pool.tile([P, P], F32)
        nc.vector.tensor_tensor(out=ee3[:, :], in0=ee2[:, :], in1=pen_m[:, :],
                                op=OP.add)
        gmat = pool.tile([P, P], F32)
        nc.scalar.activation(out=gmat[:, :], in_=ee3[:, :], func=AF.Exp)

        # ---- main pipeline ----
        sq = pool.tile([P, C * B], BF16)
        loud = pool.tile([P, B], BF16)
        env = pool.tile([P, B], F32)
        # le/eu/sp are chunk-sized scratch reused across chunks (ACT runs
        # them in engine order, so no cross-chunk hazard)
        le = pool.tile([P, FC], F32)
        eu = pool.tile([P, FC], F32)
        sp = pool.tile([P, FC], F32)
        gain = pool.tile([P, B], F32)
        ot = [pool.tile([P, FC], F32, name=f"ot{i}") for i in range(2)]

        # squares (ACT: waits its chunk's dma, one each)
        for c in range(C):
            for j in range(NF):
                nc.scalar.activation(out=sq[:, xs(c, j)], in_=xt[:, xs(c, j)],
                                     func=AF.Square)

        # loudness (DVE bf16 adds; waits ACT ticks)
        for j in range(NF):
            nc.vector.tensor_tensor(out=loud[:, fs(j)], in0=sq[:, xs(0, j)],
                                    in1=sq[:, xs(1, j)], op=OP.add)

        # zero-state scan for row-end states; all chunks overwrite one PSUM
        # scratch region (only the final column is consumed)
        a_bc = alpha[:, 0:1].to_broadcast((P, FC))
        scr_ps = ppool.tile([P, FC], F32)
        for j in range(NF):
            init = 0.0 if j == 0 else scr_ps[:, FC - 1:FC]
            nc.vector.tensor_tensor_scan(out=scr_ps[:, :], data0=a_bc,
                                         data1=loud[:, fs(j)], initial=init,
                                         op0=OP.mult, op1=OP.add)

        # carries c[p] via G^T @ ends (PE); ends copied on ACT so the
        # matmul's operands share one producer engine (one sync wait)
        ends_sb = pool.tile([P, 1], F32)
        nc.scalar.copy(ends_sb[:, :], scr_ps[:, FC - 1:FC])
        c_ps = ppool.tile([P, 1], F32)
        mm_inst = nc.tensor.matmul(c_ps[:, :], gmat[:, :], ends_sb[:, :],
                                   start=True, stop=True)

        # full scan with carry initial state
        for j in range(NF):
            init = c_ps[:, 0:1] if j == 0 else env[:, j * FC - 1:j * FC]
            nc.vector.tensor_tensor_scan(out=env[:, fs(j)], data0=a_bc,
                                         data1=loud[:, fs(j)], initial=init,
                                         op0=OP.mult, op1=OP.add)

        # gain chain on ACT (Ln/Exp only: one activation-table set)
        for j in range(NF):
            nc.scalar.activation(out=le[:, :], in_=env[:, fs(j)], func=AF.Ln,
                                 bias=ceps[:, :], scale=sclloud[:, :])
            nc.scalar.activation(out=eu[:, :], in_=le[:, :], func=AF.Exp,
                                 bias=negkT[:, :], scale=knee[:, :])
            nc.scalar.activation(out=sp[:, :], in_=eu[:, :], func=AF.Ln,
                                 bias=c1[:, :])
            gain_inst = nc.scalar.activation(out=gain[:, fs(j)],
                                             in_=sp[:, :], func=AF.Exp,
                                             scale=gamma[:, :])

        # apply gain (DVE; xt queues pre-touched, so one ACT wait each) and
        # store, double-buffering the chunk-sized output staging tiles
        k = 0
        store_insts = []
        st_touch = [pool.tile([P, 1], F32, name=f"stt{i}") for i in range(4)]
        for c in range(C):
            for j in range(NF):
                buf = ot[k % 2]
                if k >= 2:
                    # absorb the store-queue WAR wait on a separate DVE op so
                    # the mul itself carries only the ACT (gain) wait
                    ti = nc.vector.tensor_copy(st_touch[k][:, :], cm1[:, :])
                    add_dep_helper(ti.ins, store_insts[k - 2].ins, sync=True,
                                   reason="absorb store queue wait")
                mi = nc.vector.tensor_tensor(out=buf[:, :],
                                             in0=xt[:, xs(c, j)],
                                             in1=gain[:, fs(j)], op=OP.mult)
                mul_inst = mi
                add_dep_helper(mi.ins, touches[(c, j)].ins, sync=False,
                               reason="touch absorbs xt queue wait")
                si = nc.gpsimd.dma_start(out=o_view[c][:, :, fs(j)],
                                         in_=buf[:, :])
                store_insts.append(si)
                dma_insts.append(si)
                k += 1

        for di in dma_insts + [piot_inst, mm_inst, gain_inst, mul_inst]:
            ni = nc.sync.nop(nofuse=True, hint="drain_wait_absorber")
            add_dep_helper(ni.ins, di.ins, sync=True,
                           reason="absorb wait before tail drain")

    return nc


def check_waits(nc, limit=1):
    bad = []
    for b in nc.m.functions[0].blocks:
        for i in b.instructions:
            si = i.sync_info
            nw = len(si.on_wait) if si else 0
            tn = type(i).__name__
            if nw > limit and tn not in ("InstDrain", "InstEventSemOp",
                                         "InstSemaphoreOp"):
                bad.append((tn, i.name, i.engine, nw))
    return bad


def kernel(input_signals, z_alpha, log_threshold, log_ratio, log_knee):
    global _NC_CACHE
    if _NC_CACHE is None:
        _NC_CACHE = _build_nc()
    nc = _NC_CACHE

    x = np.ascontiguousarray(input_signals, dtype=np.float32)
    za = np.ascontiguousarray(z_alpha, dtype=np.float32)
    lt = np.ascontiguousarray(log_threshold, dtype=np.float32)
    lr = np.ascontiguousarray(log_ratio, dtype=np.float32)
    lk = np.ascontiguousarray(log_knee, dtype=np.float32)

    in_maps = []
    for i in range(NCORES):
        s = slice(i * NB, (i + 1) * NB)
        in_maps.append({
            "input_signals": np.ascontiguousarray(x[s]),
            "z_alpha": np.ascontiguousarray(za[s]),
            "log_threshold": np.ascontiguousarray(lt[s]),
            "log_ratio": np.ascontiguousarray(lr[s]),
            "log_knee": np.ascontiguousarray(lk[s]),
        })

    res = run_bass_kernel_spmd(nc, in_maps, core_ids=list(range(NCORES)))
    out = np.concatenate([res.results[i]["out"] for i in range(NCORES)], axis=0)
    return out.astype(np.float32, copy=False)


if __name__ == "__main__":
    nc = _build_nc()
    bad = check_waits(nc)
    print("instructions exceeding 1 wait:", bad if bad else "none")
    if "--check-only" in sys.argv:
        sys.exit(0)
    _NC_CACHE = nc
    rng = np.random.default_rng(0)
    ins = {
        "input_signals": rng.standard_normal((N, C, L)).astype(np.float32),
        "z_alpha": rng.standard_normal((N, 1)).astype(np.float32),
        "log_threshold": rng.standard_normal((N, 1)).astype(np.float32),
        "log_ratio": rng.standard_normal((N, 1)).astype(np.float32),
        "log_knee": rng.standard_normal((N, 1)).astype(np.float32),
    }
    o = kernel(**ins)
    print("kernel ran, out shape", o.shape, o.dtype)
